# revision 1
# baseline (speedup 1.0000x reference)
"""Adaptive frequency reassemble kernel for 8 TRN2 NeuronCores.

Sharding: pure data parallel over (B, D): core i owns batch b=i//4 and
d-slab [8*(i%4), 8*(i%4)+8) -> 32768 positions/core.  x_lf / x_hf are
stacked into one [128, 32768] tensor per core (lf channels on partitions
0-63, hf on 64-127).

Algebraic folds (host, exact):
  tok_t  = tokens @ W_t2f.T + b_t2f
  M      = (tok_t @ W_delta.T) * scale
  G      = M @ W_gate.T                  [8, 64]
  bg2    = W_gate @ (b_delta*scale) + b_gate
so   gate = sigmoid(G.T @ softmax_weights + bg2), and
  base   = Wsel.T @ xs  with Wsel = [diag(w_lf); diag(w_hf)]  (one matmul).

The SE-gate context (global per-(b,channel) mean) is computed ON DEVICE:
per-tile row-sums on the GPSIMD engine fused with the f32->bf16 convert,
then a tiny [128] AllReduce across the 4 cores sharing each batch, then
the 2-layer gate MLP on device.  Phase B recomputes gate from the cached
normalized attention weights (en_all, bf16 resident) and the resident
bf16 copy of xs, so each input byte is DMA'd exactly once.

Attention sub-tiles are packed 2x at PE quadrant offsets {0,32} so
exp/reciprocal/normalize run on half the free-dim width.
"""

import sys

import numpy as np

if "/opt/trn_rl_repo" not in sys.path:
    sys.path.insert(0, "/opt/trn_rl_repo")

_B, _C, _D, _H, _W = 2, 64, 32, 64, 64
_K = 8
_NCORES = 8
_NPOS = (_B * _D // _NCORES) * _H * _W  # 32768 positions per core
_NT = 1024  # phase A tile width
_NS = 512   # sub-tile width (pack factor 2)
_NTB = 1024  # phase B tile width

_NC_CACHE = {}


def _build_nc(repeat=1, no_cc=False):
    import concourse.bass as bass
    import concourse.bacc as bacc
    import concourse.mybir as mybir
    from concourse import tile
    from concourse.alu_op_type import AluOpType

    f32 = mybir.dt.float32
    bf16 = mybir.dt.bfloat16
    AF = mybir.ActivationFunctionType

    nc = bacc.Bacc(None, num_devices=1 if no_cc else _NCORES)

    xs_d = nc.declare_dram_parameter("xs", [128, _NPOS], f32, isOutput=False)
    # params packed into two blocks: one DMA + one bf16 convert instead of
    # nine DMAs + four converts at kernel start
    pbf_d = nc.declare_dram_parameter("pbf", [128, 162], f32, isOutput=False)
    pf_d = nc.declare_dram_parameter("pf32", [128, 209], f32, isOutput=False)
    out_d = nc.declare_dram_parameter("out", [64, _NPOS], f32, isOutput=True)

    cc_in = nc.dram_tensor("cc_in", [128, 1], f32)
    cc_out = nc.dram_tensor("cc_out", [128, 1], f32)
    cc_in2 = nc.dram_tensor("cc_in2", [128, 1], f32)
    cc_out2 = nc.dram_tensor("cc_out2", [128, 1], f32)

    ntiles = _NPOS // _NT  # 32
    nsub = _NT // _NS      # 2

    rep_range = range(repeat)
    with tile.TileContext(nc) as tc:
        with (
            tc.tile_pool(name="const", bufs=1) as cpool,
            tc.tile_pool(name="res", bufs=1) as rpool,
            tc.tile_pool(name="sx", bufs=8) as sxpool,
            tc.tile_pool(name="work", bufs=8) as wpool,
        ):
            # param loads ride the idle ACT sequencer so the SP queue
            # head belongs to the input stream from cycle zero
            pbf_s = cpool.tile([128, 162], f32)
            nc.scalar.dma_start(pbf_s[:], pbf_d[:])
            pf_s = cpool.tile([128, 209], f32)
            nc.scalar.dma_start(pf_s[:], pf_d[:])
            pbf16 = cpool.tile([128, 162], bf16)
            nc.vector.tensor_copy(pbf16[:], pbf_s[:])
            # views into the packed blocks
            tokbf = pbf16[:, 0:32]
            b2bf = pbf16[0:64, 32:34]
            bt2bf = pbf16[0:34, 34:98]
            g2bf = pbf16[0:64, 98:162]
            wst_s = pf_s[:, 0:16]
            wglf_s = pf_s[0:16, 16:80]
            wghf_s = pf_s[0:16, 80:144]
            i2_s = pf_s[:, 144:208]
            bg2_s = pf_s[:, 208:209]

            for _rep in rep_range:
                sxbf = rpool.tile([128, _NPOS], bf16)  # resident x (64 KB/part)
                # resident pre-sigmoid gate, 2-tile packed; sigmoid applies in
                # phase B so phase A's ACT runs only Exp/Copy (one table set)
                psg_all = rpool.tile([128, _NPOS // 2], bf16)  # 32 KB/part
                rs_cols = rpool.tile([128, ntiles], f32)       # per-tile row sums

                # ---- Phase A: stream x, attention weights + context partials ----
                psa_ctx = tc.tile_pool(name="psA", bufs=2, space="PSUM")
                with psa_ctx as psa:
                    for s in range(ntiles // 2):
                        pair_E = []
                        psD2 = psa.tile([34, _NS], f32, tag="psD2",
                                        name="psD2", bufs=1)
                        for h in range(2):
                            t = 2 * s + h
                            sl = slice(t * _NT, (t + 1) * _NT)
                            sx = sxpool.tile([128, _NT], f32, tag="sx")
                            nc.sync.dma_start(sx[:], xs_d[:, sl])
                            # f32->bf16 convert fused with row-sum partial
                            nc.vector.tensor_scalar(
                                sxbf[:, sl], sx[:], 1.0, 0.0,
                                AluOpType.mult, AluOpType.add,
                                accum_out=rs_cols[:, t:t + 1],
                            )
                        # scores for both tiles back to back (one tok LDW);
                        # token cols 8-31 are zero so rows 8-31 of bands are 0
                        psS_pair = []
                        for h in range(2):
                            t = 2 * s + h
                            psS = psa.tile([64, _NS], f32, tag="psS",
                                           name="psS")
                            for j in range(nsub):
                                nc.tensor.matmul(
                                    psS[32 * j:32 * (j + 1), :], tokbf,
                                    sxbf[:, t * _NT + j * _NS:
                                         t * _NT + (j + 1) * _NS],
                                    start=True, stop=True,
                                )
                            psS_pair.append(psS)
                        for h in range(2):
                            E = wpool.tile([64, _NS], bf16, tag="E")
                            nc.scalar.activation(E[:], psS_pair[h][:], AF.Exp)
                            pair_E.append(E)
                        # pair denominators share one psum tile (bases 0/32)
                        for h in range(2):
                            nc.tensor.matmul(psD2[32 * h:32 * h + 2, :],
                                             b2bf, pair_E[h][:],
                                             start=True, stop=True)
                        R2 = wpool.tile([34, _NS], bf16, tag="R2")
                        with nc.allow_low_precision(
                                "softmax recip in bf16: 0.4% on attn "
                                "weights, far under the 2e-2 gate"):
                            nc.vector.reciprocal(R2[:], psD2[:])
                        psG = psa.tile([128, 2 * _NS], f32, tag="psG")
                        pair_En = []
                        for hh in range(2):
                            psRB = psa.tile([64, _NS], f32, tag="psRB",
                                            name="psRB", bufs=1)
                            nc.tensor.matmul(
                                psRB[:], bt2bf[32 * hh:32 * hh + 2, :],
                                R2[32 * hh:32 * hh + 2, :],
                                start=True, stop=True,
                            )
                            En = wpool.tile([64, _NS], bf16, tag="En")
                            nc.vector.tensor_tensor(
                                out=En[:], in0=pair_E[hh][:],
                                in1=psRB[:], op=AluOpType.mult,
                            )
                            pair_En.append(En)
                        # gate matmuls grouped by G2 band (one LDW per band)
                        for j in range(2):
                            for hh in range(2):
                                nc.tensor.matmul(
                                    psG[64 * hh:64 * hh + 64,
                                        j * _NS:(j + 1) * _NS],
                                    g2bf[32 * j:32 * j + 8, :],
                                    pair_En[hh][32 * j:32 * j + 8, :],
                                    start=True, stop=True,
                                )
                        # spill pre-sigmoid gate (Copy shares Exp's table set)
                        nc.scalar.activation(
                            psg_all[:, 1024 * s:1024 * (s + 1)], psG[:],
                            AF.Copy,
                        )
                        if s == ntiles // 4 - 1 and not no_cc:
                            # first-half context partial: its AllReduce
                            # latency hides under the second half of phase A
                            rs_a = rpool.tile([128, 1], f32)
                            nc.vector.tensor_reduce(
                                rs_a[:], rs_cols[:, 0:ntiles // 2],
                                axis=mybir.AxisListType.X, op=AluOpType.add,
                            )
                            nc.sync.dma_start(cc_in[:], rs_a[:])
                            nc.gpsimd.collective_compute(
                                "AllReduce", AluOpType.add,
                                replica_groups=[[0, 1, 2, 3], [4, 5, 6, 7]],
                                ins=[cc_in[:]], outs=[cc_out[:]],
                            )
                            # fetch the first half's result mid-phase-A so
                            # only the second collective sits on the tail
                            cc_sb = rpool.tile([128, 2], f32)
                            nc.sync.dma_start(cc_sb[:, 0:1], cc_out[:])

                    # ---- second-half context AllReduce + gate MLP ----
                    rs = rpool.tile([128, 1], f32)
                    nc.vector.tensor_reduce(
                        rs[:], rs_cols[:, ntiles // 2:ntiles],
                        axis=mybir.AxisListType.X, op=AluOpType.add,
                    )
                    if no_cc:
                        ctxs = rs
                    else:
                        nc.sync.dma_start(cc_in2[:], rs[:])
                        nc.gpsimd.collective_compute(
                            "AllReduce", AluOpType.add,
                            replica_groups=[[0, 1, 2, 3], [4, 5, 6, 7]],
                            ins=[cc_in2[:]], outs=[cc_out2[:]],
                        )
                        nc.sync.dma_start(cc_sb[:, 1:2], cc_out2[:])
                        ctxs = rpool.tile([128, 1], f32)
                        nc.vector.tensor_reduce(
                            ctxs[:], cc_sb[:], axis=mybir.AxisListType.X,
                            op=AluOpType.add,
                        )
                    # MLP psums live in the phase-A pool's two free banks
                    # (fresh banks -> no WAR deps on the matmuls)
                    ps1 = psa.tile([16, 1], f32, tag="psS", name="ps1")
                    nc.tensor.matmul(ps1[:], wst_s, ctxs[:], start=True, stop=True)
                    sh = rpool.tile([16, 1], f32)
                    nc.scalar.activation(sh[:], ps1[:], AF.Relu)
                    ps2 = psa.tile([64, 1], f32, tag="psRB", name="ps2", bufs=1)
                    nc.tensor.matmul(ps2[:], wglf_s, sh[:], start=True, stop=True)
                    ps3 = psa.tile([64, 1], f32, tag="psS", name="ps3")
                    nc.tensor.matmul(ps3[:], wghf_s, sh[:], start=True, stop=True)
                    wvec = rpool.tile([128, 1], f32)
                    nc.scalar.activation(wvec[0:64, :], ps2[:], AF.Sigmoid)
                    nc.scalar.activation(wvec[64:128, :], ps3[:], AF.Sigmoid)
                    # Wsel = [diag(2*sig_lf); diag(2*sig_hf)] (the *2 is baked in I2)
                    wsel = rpool.tile([128, 64], bf16)
                    nc.vector.tensor_scalar(
                        wsel[:], i2_s, wvec[:, 0:1], None, AluOpType.mult,
                    )
                # ---- Phase B: base matmul + combine, stream out ----
                nsup = _NPOS // (2 * _NTB)  # 16
                with (
                    tc.tile_pool(name="psB", bufs=3, space="PSUM") as psbp,
                    tc.tile_pool(name="outp", bufs=6) as opool,
                ):
                    for s in reversed(range(nsup)):
                        psB = psbp.tile([128, _NTB], f32, tag="psB")
                        for h in range(2):
                            for j in range(2):
                                o0 = 2048 * s + 1024 * h + 512 * j
                                nc.tensor.matmul(
                                    psB[64 * h:64 * h + 64,
                                        j * _NS:(j + 1) * _NS],
                                    wsel[:], sxbf[:, o0:o0 + _NS],
                                    start=True, stop=True,
                                )
                        gat = opool.tile([128, _NTB], f32, tag="gat")
                        nc.scalar.activation(
                            gat[:], psg_all[:, 1024 * s:1024 * (s + 1)],
                            AF.Sigmoid, bias=bg2_s,
                        )
                        outt = opool.tile([128, _NTB], f32, tag="outt")
                        nc.vector.scalar_tensor_tensor(
                            outt[:], gat[:], 1.0,
                            psB[:], AluOpType.add, AluOpType.mult,
                        )
                        for h in range(2):
                            o0 = 2048 * s + 1024 * h
                            nc.sync.dma_start(
                                out_d[:, o0:o0 + _NTB],
                                outt[64 * h:64 * h + 64, :],
                            )

    nc.compile()
    nc.finalize()
    return nc


def _get_nc(repeat=1, no_cc=False):
    key = f"nc{repeat}_{no_cc}"
    if key not in _NC_CACHE:
        _NC_CACHE[key] = _build_nc(repeat, no_cc)
    return _NC_CACHE[key]


def _host_params(inputs):
    f = np.float32
    tokens = np.asarray(inputs["tokens"], f)
    scale = float(np.asarray(inputs["scale"]).reshape(-1)[0])
    sf = _C ** -0.5
    tok32 = np.zeros((128, 32), f)
    tok32[0:64, 0:_K] = tokens.T * sf
    tok32[64:128, 0:_K] = tokens.T * sf
    tok_t = tokens @ np.asarray(inputs["W_t2f"], f).T + np.asarray(inputs["b_t2f"], f)
    M = (tok_t @ np.asarray(inputs["W_delta"], f).T) * scale
    W_gate = np.asarray(inputs["W_gate"], f)
    G = M @ W_gate.T  # [8, 64]
    bg2v = (W_gate @ (np.asarray(inputs["b_delta"], f) * scale)
            + np.asarray(inputs["b_gate"], f))
    bg2 = np.concatenate([bg2v, bg2v])[:, None]  # stacked for both halves
    # quadrant-packed selector / replication matrices (bands at 0 and 32)
    B2 = np.zeros((64, 2), f)
    Bt2 = np.zeros((34, 64), f)
    G2 = np.zeros((64, 64), f)
    for j in range(2):
        B2[32 * j:32 * j + 8, j] = 1.0
        # replicated at partition bases 0 and 32 for the pair-batched recip
        Bt2[j, 32 * j:32 * j + 8] = 1.0
        Bt2[32 + j, 32 * j:32 * j + 8] = 1.0
        G2[32 * j:32 * j + 8, :] = G
    WsT = np.ascontiguousarray(
        np.asarray(inputs["W_shared"], f).T / (_D * _H * _W))
    WglfT = np.ascontiguousarray(np.asarray(inputs["W_glf"], f).T)
    WghfT = np.ascontiguousarray(np.asarray(inputs["W_ghf"], f).T)
    eye2 = np.eye(64, dtype=f) * 2.0
    I2 = np.ascontiguousarray(np.concatenate([eye2, eye2], 0))
    pbf = np.zeros((128, 162), f)
    pbf[:, 0:32] = tok32
    pbf[0:64, 32:34] = B2
    pbf[0:34, 34:98] = Bt2
    pbf[0:64, 98:162] = G2
    pf32 = np.zeros((128, 209), f)
    pf32[:, 0:16] = WsT
    pf32[0:16, 16:80] = WglfT
    pf32[0:16, 80:144] = WghfT
    pf32[:, 144:208] = I2
    pf32[:, 208:209] = bg2
    return {"pbf": pbf, "pf32": pf32}


def kernel(**inputs):
    from concourse.bass_utils import run_bass_kernel_spmd

    x_hf = np.asarray(inputs["x_hf"], np.float32)
    x_lf = np.asarray(inputs["x_lf"], np.float32)
    params = _host_params(inputs)

    in_maps = []
    for i in range(_NCORES):
        b, d0 = i // 4, 8 * (i % 4)
        xl = x_lf[b, :, d0:d0 + 8].reshape(64, -1)
        xh = x_hf[b, :, d0:d0 + 8].reshape(64, -1)
        xs = np.ascontiguousarray(np.concatenate([xl, xh], 0), np.float32)
        m = {"xs": xs}
        m.update(params)
        in_maps.append(m)

    nc = _get_nc()
    res = run_bass_kernel_spmd(nc, in_maps, list(range(_NCORES)))
    out = np.empty((_B, _C, _D, _H, _W), np.float32)
    for i in range(_NCORES):
        b, d0 = i // 4, 8 * (i % 4)
        out[b, :, d0:d0 + 8] = np.asarray(res.results[i]["out"]).reshape(
            64, 8, _H, _W)
    return out

